# revision 6
# baseline (speedup 1.0000x reference)
"""Causal self-attention (B=2, T=2048, D=2048, H=16, HD=128) on 8 TRN2 cores.

Strategy: tensor-parallel over heads (2 heads/core) for QKV projection, RMS
norm, rotary, and attention; AllToAll reshards the attention output over
tokens; each core then runs the output projection for its 512-token slice.
All matmul contractions sit on the partition dim via host-side transposes:
  - qT/kT/vT come out of the QKV matmul as [outdim, token] tiles directly,
  - scores are computed transposed (S^T = krot^T.T @ qrot^T -> [k, q]), so
    the PV matmul needs no on-device transposes,
  - softmax denominator = all-ones matmul over expS^T (also acts as the
    partition-broadcast), normalization folds into the PSUM->SBUF copy.
Matmuls run in bf16 (fp32 is 4 cycles/row on the PE, bf16 is 1); PSUM
accumulation and softmax statistics stay fp32.
"""

import numpy as np

B, T, D = 2, 2048, 2048
H, HD = 16, 128
N_CORES = 8
HPC = H // N_CORES          # heads per core
NT = B * T                  # 4096 tokens, b-major
TS = NT // N_CORES          # 512-token output slice per core
DC = D // 128               # 16 contraction chunks
NTT = NT // 512             # 8 token tiles in phase 1
KT_PER_B = T // 128         # 16 k-tiles per batch row

_CACHE = {}


def _build(scale: float):
    import concourse.bacc as bacc
    import concourse.mybir as mybir
    import concourse.tile as tile

    f32 = mybir.dt.float32
    MM = mybir.dt.bfloat16
    EPS = float(np.finfo(np.float32).eps)

    nc = bacc.Bacc("TRN2", target_bir_lowering=False, debug=False,
                   num_devices=N_CORES)

    xT_d = nc.dram_tensor("xT", [D, NT], MM, kind="ExternalInput")
    wqk_d = nc.dram_tensor("wqk", [D, 4 * 128], MM, kind="ExternalInput")
    wv_d = nc.dram_tensor("wv", [D, HPC * HD], MM, kind="ExternalInput")
    wo_d = nc.dram_tensor("wo", [D, D], MM, kind="ExternalInput")
    cs_d = nc.dram_tensor("cs", [2, HD, NT], MM, kind="ExternalInput")
    mask_d = nc.dram_tensor("mask", [4, 128, 512], MM, kind="ExternalInput")
    ones_d = nc.dram_tensor("ones", [128, 128], MM, kind="ExternalInput")
    y_d = nc.dram_tensor("y", [TS, D], f32, kind="ExternalOutput")

    Sq = mybir.ActivationFunctionType.Square
    Sqrt = mybir.ActivationFunctionType.Sqrt
    Exp = mybir.ActivationFunctionType.Exp
    Copy = mybir.ActivationFunctionType.Copy
    mult = mybir.AluOpType.mult
    add = mybir.AluOpType.add

    with tile.TileContext(nc) as tc:
        with tc.tile_pool(name="dram", bufs=1, space="DRAM") as dram, \
             tc.tile_pool(name="res", bufs=1) as res:
            a2a_in = dram.tile([N_CORES, HPC * HD, TS], MM, tag="a2a_in")
            a2a_out = dram.tile([N_CORES, HPC * HD, TS], MM, tag="a2a_out")

            # Residents through phase 2: rotated q/k (m-chunks: q0,q1,k0,k1),
            # v in [token, hd] layout, causal masks, all-ones stationary.
            qk_sb = res.tile([128, 4 * NT], MM, tag="qk")
            v_sb = res.tile([128, (NT // 128) * (HPC * HD)], MM, tag="v")
            mask_sb = res.tile([128, 4 * 512], MM, tag="mask")
            ones_sb = res.tile([128, 128], MM, tag="ones")
            eps_sb = res.tile([128, 1], f32, tag="eps")
            nc.vector.memset(eps_sb[:], EPS)
            for m in range(4):
                nc.sync.dma_start(out=mask_sb[:, m * 512:(m + 1) * 512],
                                  in_=mask_d[m, :, :])
            nc.sync.dma_start(out=ones_sb[:], in_=ones_d[:, :])

            # ---------------- Phase 1: QKV + RMS norm + rotary ----------------
            with tc.tile_pool(name="p1", bufs=1) as p1, \
                 tc.tile_pool(name="xs", bufs=2) as xs, \
                 tc.tile_pool(name="st", bufs=2) as st, \
                 tc.tile_pool(name="ps1", bufs=2, space="PSUM") as ps1:
                wqk_sb = p1.tile([128, DC * 512], MM, tag="wqk")
                wv_sb = p1.tile([128, DC * HPC * HD], MM, tag="wv")
                cs_sb = p1.tile([128, 2 * NT], MM, tag="cs")
                nc.sync.dma_start(
                    out=wqk_sb[:].rearrange("p (c f) -> p c f", f=512),
                    in_=wqk_d[:, :].rearrange("(c p) f -> p c f", p=128))
                nc.sync.dma_start(
                    out=wv_sb[:].rearrange("p (c f) -> p c f", f=256),
                    in_=wv_d[:, :].rearrange("(c p) f -> p c f", p=128))
                for s in range(2):
                    nc.sync.dma_start(out=cs_sb[:, s * NT:(s + 1) * NT],
                                      in_=cs_d[s, :, :])

                for n in range(NTT):
                    xblk = xs.tile([128, DC * 512], MM, tag="xblk")
                    for cg in range(4):
                        nc.sync.dma_start(
                            out=xblk[:, cg * 4 * 512:(cg + 1) * 4 * 512]
                                .rearrange("p (c f) -> p c f", f=512),
                            in_=xT_d[cg * 512:(cg + 1) * 512, n * 512:(n + 1) * 512]
                                .rearrange("(c p) f -> p c f", p=128))
                    # v projection: [token, hd] layout
                    for c4 in range(4):
                        vps = ps1.tile([128, HPC * HD], f32, tag="vps")
                        for dc in range(DC):
                            nc.tensor.matmul(
                                vps[:],
                                xblk[:, dc * 512 + c4 * 128: dc * 512 + (c4 + 1) * 128],
                                wv_sb[:, dc * 256:(dc + 1) * 256],
                                start=(dc == 0), stop=(dc == DC - 1))
                        tcg = n * 4 + c4
                        nc.vector.tensor_copy(v_sb[:, tcg * 256:(tcg + 1) * 256], vps[:])
                    # q/k projection + rms + rotary, m-chunks q0,q1,k0,k1
                    for m in range(4):
                        qps = ps1.tile([128, 512], f32, tag="qps")
                        for dc in range(DC):
                            nc.tensor.matmul(
                                qps[:],
                                wqk_sb[:, dc * 512 + m * 128: dc * 512 + (m + 1) * 128],
                                xblk[:, dc * 512:(dc + 1) * 512],
                                start=(dc == 0), stop=(dc == DC - 1))
                        sq = st.tile([128, 512], MM, tag="sq")
                        nc.scalar.activation(sq[:], qps[:], Sq)
                        ssq = ps1.tile([128, 512], f32, tag="ssq")
                        nc.tensor.matmul(ssq[:], ones_sb[:], sq[:], start=True, stop=True)
                        rms = st.tile([128, 512], f32, tag="rms")
                        nc.scalar.activation(rms[:], ssq[:], Sqrt, bias=eps_sb[:], scale=1.0 / HD)
                        r = st.tile([128, 512], f32, tag="r")
                        nc.vector.reciprocal(r[:], rms[:])
                        qn = st.tile([128, 512], MM, tag="qn")
                        nc.vector.tensor_mul(qn[:], qps[:], r[:])
                        # rotary: y = qn*C + swap(qn)*S  with S = [sin; -sin]
                        tsw = st.tile([128, 512], MM, tag="tsw")
                        ctile = cs_sb[:, n * 512:(n + 1) * 512]
                        stile = cs_sb[:, NT + n * 512: NT + (n + 1) * 512]
                        # stile holds [-sin; sin]: each mul's inputs share a
                        # base partition; only the output is partition-shifted.
                        nc.vector.tensor_mul(tsw[0:64, :], qn[64:128, :], stile[64:128, :])
                        nc.vector.tensor_mul(tsw[64:128, :], qn[0:64, :], stile[0:64, :])
                        dst = qk_sb[:, m * NT + n * 512: m * NT + (n + 1) * 512]
                        nc.vector.tensor_mul(dst, qn[:], ctile)
                        nc.vector.tensor_add(dst, dst, tsw[:])

            # ---------------- Phase 2: causal attention ----------------
            with tc.tile_pool(name="p2", bufs=3) as p2, \
                 tc.tile_pool(name="p2b", bufs=2) as p2b, \
                 tc.tile_pool(name="pss", bufs=2, space="PSUM") as pss, \
                 tc.tile_pool(name="psd", bufs=2, space="PSUM") as psd, \
                 tc.tile_pool(name="psy", bufs=2, space="PSUM") as psy:
                for b in range(B):
                    for h in range(HPC):
                        qoff = h * NT + b * T
                        koff = (2 + h) * NT + b * T
                        for qj in range(4):
                            yps = psy.tile([128, 512], f32, tag="yps")
                            dps = psd.tile([128, 512], f32, tag="dps")
                            nkt = 4 * qj + 4
                            for kb in range(nkt):
                                sps = pss.tile([128, 512], f32, tag="sps")
                                nc.tensor.matmul(
                                    sps[:],
                                    qk_sb[:, koff + kb * 128: koff + (kb + 1) * 128],
                                    qk_sb[:, qoff + qj * 512: qoff + (qj + 1) * 512],
                                    start=True, stop=True)
                                e = p2.tile([128, 512], MM, tag="e")
                                nc.scalar.activation(e[:], sps[:], Exp, scale=scale)
                                if kb >= 4 * qj:
                                    mi = kb - 4 * qj
                                    nc.vector.tensor_mul(
                                        e[:], e[:], mask_sb[:, mi * 512:(mi + 1) * 512])
                                nc.tensor.matmul(dps[:], ones_sb[:], e[:],
                                                 start=(kb == 0), stop=(kb == nkt - 1))
                                tcg = b * KT_PER_B + kb
                                nc.tensor.matmul(
                                    yps[:],
                                    v_sb[:, tcg * 256 + h * 128: tcg * 256 + (h + 1) * 128],
                                    e[:],
                                    start=(kb == 0), stop=(kb == nkt - 1))
                            rcp = p2b.tile([128, 512], f32, tag="rcp")
                            nc.vector.reciprocal(rcp[:], dps[:])
                            yn = p2b.tile([128, 512], MM, tag="yn")
                            nc.vector.tensor_mul(yn[:], yps[:], rcp[:])
                            s = b * 4 + qj
                            nc.sync.dma_start(
                                out=a2a_in[s, h * 128:(h + 1) * 128, :], in_=yn[:])

            nc.gpsimd.collective_compute(
                "AllToAll",
                mybir.AluOpType.bypass,
                replica_groups=[list(range(N_CORES))],
                ins=[a2a_in.opt()],
                outs=[a2a_out.opt()],
            )

            # ---------------- Phase 3: output projection ----------------
            with tc.tile_pool(name="p3", bufs=1) as p3, \
                 tc.tile_pool(name="wop", bufs=2) as wop, \
                 tc.tile_pool(name="ob", bufs=2) as obp, \
                 tc.tile_pool(name="ps3", bufs=2, space="PSUM") as ps3:
                yT_sb = p3.tile([128, DC * 512], MM, tag="yT")
                nc.sync.dma_start(
                    out=yT_sb[:].rearrange("p (c f) -> p c f", f=512),
                    in_=a2a_out.rearrange("g r f -> (g r) f")
                               .rearrange("(c p) f -> p c f", p=128))
                for on in range(4):
                    wo_sb = wop.tile([128, DC * 512], MM, tag="wo")
                    for cg in range(4):
                        nc.sync.dma_start(
                            out=wo_sb[:, cg * 4 * 512:(cg + 1) * 4 * 512]
                                .rearrange("p (c f) -> p c f", f=512),
                            in_=wo_d[cg * 512:(cg + 1) * 512, on * 512:(on + 1) * 512]
                                .rearrange("(c p) f -> p c f", p=128))
                    for mc in range(4):
                        ops_ = ps3.tile([128, 512], f32, tag="ops")
                        for dc in range(DC):
                            nc.tensor.matmul(
                                ops_[:],
                                yT_sb[:, dc * 512 + mc * 128: dc * 512 + (mc + 1) * 128],
                                wo_sb[:, dc * 512:(dc + 1) * 512],
                                start=(dc == 0), stop=(dc == DC - 1))
                        ob = obp.tile([128, 512], f32, tag="ob")
                        nc.scalar.activation(ob[:], ops_[:], Copy)
                        nc.sync.dma_start(
                            out=y_d[mc * 128:(mc + 1) * 128, on * 512:(on + 1) * 512],
                            in_=ob[:])

    nc.compile()
    return nc


def _prep_inputs(x, W, cos, sin):
    import concourse.mybir as mybir
    bf = mybir.dt.np(mybir.dt.bfloat16)

    xT = np.ascontiguousarray(x.reshape(NT, D).T).astype(bf)
    cT = cos.T.astype(np.float32)
    sT = sin.T.astype(np.float32)
    C128 = np.tile(np.concatenate([cT, cT], 0), (1, B)).astype(bf)
    S128 = np.tile(np.concatenate([-sT, sT], 0), (1, B)).astype(bf)
    cs = np.ascontiguousarray(np.stack([C128, S128]))
    masks = np.stack([
        (np.arange(128)[:, None] <= np.arange(512)[None, :] - 128 * m)
        for m in range(4)
    ]).astype(bf)
    ones = np.ones((128, 128), dtype=bf)
    wo = np.ascontiguousarray(W[3].T).astype(bf)

    in_maps = []
    for c in range(N_CORES):
        r0 = c * HPC * HD
        wqk = np.ascontiguousarray(
            np.concatenate([W[0][r0:r0 + 256], W[1][r0:r0 + 256]], 0).T).astype(bf)
        wv = np.ascontiguousarray(W[2][r0:r0 + 256].T).astype(bf)
        in_maps.append({
            "xT": xT, "wqk": wqk, "wv": wv, "wo": wo,
            "cs": cs, "mask": masks, "ones": ones,
        })
    return in_maps


def kernel(x, W, cos, sin, scale):
    from concourse.bass_utils import run_bass_kernel_spmd

    x = np.asarray(x, dtype=np.float32)
    W = np.asarray(W, dtype=np.float32)
    cos = np.asarray(cos, dtype=np.float32)
    sin = np.asarray(sin, dtype=np.float32)
    sc = float(np.asarray(scale))

    if sc not in _CACHE:
        _CACHE[sc] = _build(sc)
    nc = _CACHE[sc]

    in_maps = _prep_inputs(x, W, cos, sin)
    out = run_bass_kernel_spmd(nc, in_maps, core_ids=list(range(N_CORES)))
    y = np.concatenate([out.results[c]["y"] for c in range(N_CORES)], axis=0)
    return y.reshape(B, T, D)
